# revision 15
# baseline (speedup 1.0000x reference)
"""Trainium2 Bass kernel for the HexPlane-style decoder (nn_DecoderBase).

Math (B=1): six 3x3 SAME convs (64->16ch) + bias + ReLU + 2x nearest
upsample, channels-last, then broadcast Hadamard into
voxel[t, x, y, z, c] of shape [16, 64, 64, 32, 16] (f32, 128 MiB).

Key observation: every axis of the voxel (t, x, y, z) is 2x
nearest-upsampled, so out[t,x,y,z,c] depends only on
(t//2, x//2, y//2, z//2, c) -- only 1/16 of the output is unique.
We compute just the unique block per core and let the output DMAs
duplicate it on the way to HBM.

Sharding: X (64) split across 8 cores -> 4 unique x2-values per core
(conv halos sliced host-side).  Per core, with partitions p=(x2,y2):

  out[t2,x2,y2,z2,c] = M1[p,(z2,c)] * ty[t2,y2,c] * Q[t2,x2,z2,c]
  M1 = uxy*uxz*uyz (pre-upsample conv outs),  Q = utx*utz.

All cross-partition broadcasts are done by tiny replicated DMA loads
from conv-output dumps in DRAM (0-stride partition dims), so the whole
voxel phase is a handful of VectorE tensor_tensor ops; no PE matmuls
outside the convolutions.  Each unique [128, 1024] f32 tile is stored
4x (t-dup x x-dup) with y/z duplication folded into the DMA access
patterns (4 KiB contiguous runs).
"""

import numpy as np

T2, X2, Y2, Z2, C = 8, 4, 32, 16, 16
NCORES = 8
CIN = 64

_CACHE = {}


def _build_program():
    from contextlib import ExitStack

    import concourse.bacc as bacc
    import concourse.bass as bass
    import concourse.mybir as mybir
    from concourse.tile import TileContext

    f32 = mybir.dt.float32
    AF = mybir.ActivationFunctionType
    MUL = mybir.AluOpType.mult
    AP = bass.AP

    nc = bacc.Bacc()
    ctx = ExitStack()

    # ---- external IO ----
    # One packed fp16 input: rows 0..63 = cin, row 64 = ones (bias channel).
    # Column segments: xyT[0:206] xz[206:316] yz[316:930] tx[930:992]
    # tyT[992:1334] tz[1334:1516] w[1516:2380]; convs read 3x3 windows,
    # w holds (plane, dy, dx, cout) with the bias in row 64 of the center tap.
    f16 = mybir.dt.float16
    KP = CIN + 1
    img_all = nc.dram_tensor("img_all", [KP, 2380], f16, kind="ExternalInput")
    out_d = nc.dram_tensor("out", [2 * T2, 2 * X2, 2 * Y2, 2 * Z2, C], f32,
                           kind="ExternalOutput")
    SEG = {"xyT": 0, "xz": 206, "yz": 316, "tx": 930, "tyT": 992,
           "tz": 1334, "w": 1516}

    # ---- DRAM scratch: raw conv-output dumps (flat [m*16]) ----
    yz_rows = [(0, 7), (7, 7), (14, 7), (21, 7), (28, 4)]
    ty_rows = [(0, 12), (12, 12), (24, 8)]
    edump = {}
    xy_rows = [(0, 21), (21, 11)]
    for k, m in ([("xz", 72), ("tx", 48), ("tz0", 72), ("tz1", 72)]
                 + [(f"xy{b}", nr * 6) for b, (r0, nr) in enumerate(xy_rows)]
                 + [(f"yz{b}", nr * 18) for b, (r0, nr) in enumerate(yz_rows)]
                 + [(f"ty{b}", nr * 10) for b, (r0, nr) in enumerate(ty_rows)]):
        edump[k] = nc.dram_tensor(f"e_{k}", [m * 16], f32)
    quD = nc.dram_tensor("quD", [32 * 256], f32)  # (x2, t2, z2, c) flat
    warmD = nc.dram_tensor("warmD", [16], f32)

    with TileContext(nc) as tc:
        sb = lambda name, shape: ctx.enter_context(
            nc.sbuf_tensor(name, shape, f32))
        # inputs (single packed fp16 tile)
        i_all = ctx.enter_context(nc.sbuf_tensor("i_all", [KP, 2380], f16))
        # voxel operands (partitions p = y2*4 + x2 unless noted)
        uxy_sb = sb("uxy_sb", [128, 16])      # p: c
        uxz_rep = sb("uxz_rep", [128, 256])   # p: (z2, c)  [rep over y2]
        uyz_rep = sb("uyz_rep", [128, 256])   # p: (z2, c)  [rep over x2]
        uty_rep = sb("uty_rep", [128, 128])   # p: (t2, c)  [rep over x2]
        qu_rep = sb("qu_rep", [128, 2048])    # p: (t2, z2, c)  [rep over y2]
        utx_sb = sb("utx_sb", [32, 16])       # p=(t2,x2): c
        utz_sb = sb("utz_sb", [32, 256])      # p=(t2,x2): (z2, c)
        qu_sb = sb("qu_sb", [32, 256])        # p=(t2,x2): (z2, c)
        m1a = sb("m1a", [128, 256])
        m1u = sb("m1u", [128, 256])
        tmp_all = sb("tmp_all", [128, 2048])  # p: (t2, z2, c) = m1u * ty

        # ---------- phase A: input load ----------
        nc.sync.dma_start(i_all[:], img_all[:])

        # ---------- PE warm-up (runs during the input DMA) ----------
        # The HAM clock gate keeps PE at 1.2 GHz until ~3.4us of sustained
        # activity; burn dummy matmuls so the convolutions run at 2.4 GHz.
        warm_sb = ctx.enter_context(nc.sbuf_tensor("warm_sb", [128, 512], f16))
        warm_out = sb("warm_out", [1, 16])
        nc.vector.memset(warm_sb[:], 0.0)
        with tc.tile_pool(name="warmpsum", bufs=2, space="PSUM") as wpool:
            wp_t = None
            for i in range(20):
                wp_t = wpool.tile([128, 512], f32, name=f"wp{i}", tag="wp")
                nc.tensor.matmul(wp_t, warm_sb[:, :128], warm_sb[:],
                                 start=True, stop=True)
            nc.scalar.activation(warm_out[:], wp_t[:1, :16], AF.Relu)
        nc.sync.dma_start(warmD[:], warm_out[:])

        # ---------- phase B: convolutions ----------
        def wslice(i, dy, dx):
            off = SEG["w"] + ((i * 3 + dy) * 3 + dx) * 16
            return AP(i_all, off, [[2380, KP], [1, 16]])

        conv_pool_cm = tc.tile_pool(name="convpsum", bufs=2, space="PSUM")
        conv_pool = conv_pool_cm.__enter__()

        conv_outs = {}

        def conv_spatial(i, seg, wp, rows, row0, tag):
            # Full-width contiguous windows; junk at cols wp-2, wp-1.
            m = rows * wp
            psum = conv_pool.tile([m, 16], f32, name=f"cp_{tag}", tag="cp")
            for dy in range(3):
                for dx in range(3):
                    lhsT = AP(i_all, SEG[seg] + (row0 + dy) * wp + dx,
                              [[2380, KP], [1, m]])
                    nc.tensor.matmul(psum, lhsT, wslice(i, dy, dx),
                                     start=(dy == 0 and dx == 0),
                                     stop=(dy == 2 and dx == 2))
            out_sb = sb(f"c_{tag}", [m, 16])
            nc.scalar.activation(out_sb[:], psum, AF.Relu)
            conv_outs[tag] = out_sb

        dump_insts = {}

        def dump(eng, k):
            dump_insts[k] = eng.dma_start(edump[k][:], conv_outs[k][:])

        def reload(eng, deps, dst_ap, src_ap):
            inst = eng.dma_start(dst_ap, src_ap)
            for d in deps:
                bass._add_dep_helper(inst.ins, dump_insts[d].ins,
                                     reason=f"raw {d}")
            return inst

        # --- Q path first: tx, tz, then ty ---
        conv_spatial(3, "tx", 6, 8, 0, "tx")               # m=48
        dump(nc.sync, "tx")
        for k in range(2):
            conv_spatial(5, "tz", 18, 4, 4 * k, f"tz{k}")  # m=72
            dump(nc.sync, f"tz{k}")
        for b, (r0, nr) in enumerate(ty_rows):
            conv_spatial(4, "tyT", 10, nr, r0, f"ty{b}")
            dump(nc.sync, f"ty{b}")

        # Q = utx * utz on p=(t2, x2)
        reload(nc.sync, ["tx"], utx_sb[:],
               AP(edump["tx"], 0, [[96, 8], [16, 4], [1, 16]]))
        for k in range(2):
            reload(nc.sync, [f"tz{k}"],
                   AP(utz_sb, k * 16 * 256, [[256, 16], [1, 256]]),
                   AP(edump[f"tz{k}"], 0, [[288, 4], [0, 4], [1, 256]]))
        nc.vector.tensor_tensor(
            qu_sb[:], utz_sb[:], AP(utx_sb, 0, [[16, 32], [0, 16], [1, 16]]),
            MUL)
        # store as (x2, t2, z2, c) so the replicated reload is contiguous
        qu_store = nc.sync.dma_start(
            AP(quD, 0, [[256, 8], [2048, 4], [1, 256]]), qu_sb[:])
        qu_load = nc.sync.dma_start(
            qu_rep[:], AP(quD, 0, [[0, 32], [2048, 4], [1, 2048]]))
        bass._add_dep_helper(qu_load.ins, qu_store.ins, reason="raw quD")

        # uty_rep[p=(y2,x2), (t2, c)] from transposed-ty dumps
        for b, (r0, nr) in enumerate(ty_rows):
            reload(nc.sync, [f"ty{b}"],
                   AP(uty_rep, r0 * 4 * 128, [[128, 4 * nr], [1, 128]]),
                   AP(edump[f"ty{b}"], 0, [[160, nr], [0, 4], [1, 128]]))

        # --- M1 path: xy (transposed plane -> y2-major rows), xz, yz ---
        for b, (r0, nr) in enumerate(xy_rows):
            conv_spatial(0, "xyT", 6, nr, r0, f"xy{b}")
            dump(nc.scalar, f"xy{b}")
            reload(nc.scalar, [f"xy{b}"],
                   AP(uxy_sb, r0 * 4 * 16, [[16, 4 * nr], [1, 16]]),
                   AP(edump[f"xy{b}"], 0, [[96, nr], [16, 4], [1, 16]]))

        conv_spatial(1, "xz", 18, 4, 0, "xz")             # m=72
        dump(nc.scalar, "xz")
        reload(nc.scalar, ["xz"], uxz_rep[:],
               AP(edump["xz"], 0, [[0, 32], [288, 4], [1, 256]]))
        for b, (r0, nr) in enumerate(yz_rows):
            conv_spatial(2, "yz", 18, nr, r0, f"yz{b}")
            dump(nc.scalar, f"yz{b}")
            reload(nc.scalar, [f"yz{b}"],
                   AP(uyz_rep, r0 * 4 * 256, [[256, 4 * nr], [1, 256]]),
                   AP(edump[f"yz{b}"], 0, [[288, nr], [0, 4], [1, 256]]))

        conv_pool_cm.__exit__(None, None, None)

        # ---------- phase C: M1 and ty products ----------
        nc.vector.tensor_tensor(m1a[:], uxz_rep[:], uyz_rep[:], MUL)
        nc.vector.tensor_tensor(
            m1u[:], m1a[:], AP(uxy_sb, 0, [[16, 128], [0, 16], [1, 16]]), MUL)
        # tmp_all[p, (t2, z2, c)] = m1u[p, (z2, c)] * uty_rep[p, (t2, c)]
        nc.vector.tensor_tensor(
            tmp_all[:],
            AP(m1u, 0, [[256, 128], [0, 8], [16, 16], [1, 16]]),
            AP(uty_rep, 0, [[128, 128], [16, 8], [0, 16], [1, 16]]), MUL)

        # ---------- phase D: per-t2 voxel tiles + duplicated stores ----------
        from contextlib import ExitStack as _ES
        pool_ctx = _ES()
        out_pool = pool_ctx.enter_context(tc.tile_pool(name="outsb", bufs=5))

        for t2 in range(T2):
            o = out_pool.tile([128, 1024], f32, name="o", tag="o")
            op = o.ap[0][0]
            # o[p, (z2, zd, c)] = tmp_all[p, t2, z2, c] * qu_rep[p, t2, z2, c]
            nc.vector.tensor_tensor(
                AP(o.tensor, o.offset, [[op, 128], [32, 16], [16, 2], [1, 16]]),
                AP(tmp_all, t2 * 256, [[2048, 128], [16, 16], [0, 2], [1, 16]]),
                AP(qu_rep, t2 * 256, [[2048, 128], [16, 16], [0, 2], [1, 16]]),
                MUL)
            # duplicate the (z, c) half-row for the y-duplication run
            nc.vector.tensor_copy(
                AP(o.tensor, o.offset + 512, [[op, 128], [1, 512]]),
                AP(o.tensor, o.offset, [[op, 128], [1, 512]]))
            for td in range(2):
                for xd in range(2):
                    eng = nc.sync if (td * 2 + xd) % 2 == 0 else nc.scalar
                    dst = AP(out_d,
                             (2 * t2 + td) * 262144 + xd * 32768,
                             [[1024, 32], [65536, 4], [1, 1024]])
                    eng.dma_start(dst, o[:])

        pool_ctx.close()

    nc.compile()
    return nc, ctx


def _prep_inputs(plane_xy, plane_xz, plane_yz, plane_tx, plane_ty, plane_tz,
                 W, b):
    """Host-side slicing/padding/transposition into one packed fp16 input."""
    f32 = np.float32
    xy = np.asarray(plane_xy, f32)[0]  # [64, X'32, Y'32]
    xz = np.asarray(plane_xz, f32)[0]  # [64, X'32, Z'16]
    yz = np.asarray(plane_yz, f32)[0]  # [64, Y'32, Z'16]
    tx = np.asarray(plane_tx, f32)[0]  # [64, T'8,  X'32]
    ty = np.asarray(plane_ty, f32)[0]  # [64, T'8,  Y'32]
    tz = np.asarray(plane_tz, f32)[0]  # [64, T'8,  Z'16]
    W = np.asarray(W, f32)             # [6, 16, 64, 3, 3]
    b = np.asarray(b, f32)             # [6, 16]

    # xy and ty are convolved on transposed planes -> swap their 3x3 taps
    W2 = W.copy()
    W2[0] = W[0].transpose(0, 1, 3, 2)
    W2[4] = W[4].transpose(0, 1, 3, 2)
    # weight block [65, 864]: rows 0..63 = (ci, i, dy, dx, co); row 64 holds
    # the bias in the center tap (the ones-channel contributes it once).
    wseg = np.zeros((65, 864), f32)
    wseg[:64] = W2.transpose(2, 0, 3, 4, 1).reshape(CIN, 864)
    for i in range(6):
        wseg[64, ((i * 3 + 1) * 3 + 1) * 16:((i * 3 + 1) * 3 + 1) * 16 + 16] = b[i]

    def flat2(p):
        q = p.reshape(p.shape[0], -1)
        return np.ascontiguousarray(np.pad(q, ((0, 0), (0, 2))))

    def with_ones(img):
        return np.concatenate([img, np.ones((1, img.shape[1]), f32)], axis=0)

    img_yz = flat2(np.pad(yz, ((0, 0), (1, 1), (1, 1))))
    img_tyT = flat2(np.pad(ty.transpose(0, 2, 1), ((0, 0), (1, 1), (1, 1))))
    img_tz = flat2(np.pad(tz, ((0, 0), (1, 1), (1, 1))))

    def row_halo(p, x0h):
        out = np.zeros((p.shape[0], 6, p.shape[2]), f32)
        lo = x0h - 1
        s0, s1 = max(lo, 0), min(lo + 6, p.shape[1])
        out[:, s0 - lo:s0 - lo + (s1 - s0), :] = p[:, s0:s1, :]
        return out

    def col_halo(p, x0h):
        out = np.zeros((p.shape[0], p.shape[1], 6), f32)
        lo = x0h - 1
        s0, s1 = max(lo, 0), min(lo + 6, p.shape[2])
        out[:, :, s0 - lo:s0 - lo + (s1 - s0)] = p[:, :, s0:s1]
        return out

    in_maps = []
    for k in range(NCORES):
        x0h = 4 * k
        segs = [
            flat2(np.pad(col_halo(xy.transpose(0, 2, 1), x0h),
                         ((0, 0), (1, 1), (0, 0)))),            # xyT 206
            flat2(np.pad(row_halo(xz, x0h), ((0, 0), (0, 0), (1, 1)))),  # 110
            img_yz,                                             # 614
            flat2(np.pad(col_halo(tx, x0h), ((0, 0), (1, 1), (0, 0)))),  # 62
            img_tyT,                                            # 342
            img_tz,                                             # 182
        ]
        img = np.concatenate([with_ones(s) for s in segs] + [wseg], axis=1)
        in_maps.append({"img_all": img.astype(np.float16)})
    return in_maps


def kernel(plane_xy, plane_xz, plane_yz, plane_tx, plane_ty, plane_tz, W, b):
    from concourse.bass_utils import run_bass_kernel_spmd

    if "nc" not in _CACHE:
        _CACHE["nc"], _CACHE["ctx"] = _build_program()
    nc = _CACHE["nc"]

    in_maps = _prep_inputs(plane_xy, plane_xz, plane_yz, plane_tx, plane_ty,
                           plane_tz, W, b)
    res = run_bass_kernel_spmd(nc, in_maps, list(range(NCORES)))
    slices = [res.results[k]["out"] for k in range(NCORES)]
    full = np.concatenate(slices, axis=1)  # [T, 64, Y, Z, C]
    return full[None].astype(np.float32)


# revision 16
# speedup vs baseline: 1.0372x; 1.0372x over previous
"""Trainium2 Bass kernel for the HexPlane-style decoder (nn_DecoderBase).

Math (B=1): six 3x3 SAME convs (64->16ch) + bias + ReLU + 2x nearest
upsample, channels-last, then broadcast Hadamard into
voxel[t, x, y, z, c] of shape [16, 64, 64, 32, 16] (f32, 128 MiB).

Key observation: every axis of the voxel (t, x, y, z) is 2x
nearest-upsampled, so out[t,x,y,z,c] depends only on
(t//2, x//2, y//2, z//2, c) -- only 1/16 of the output is unique.
We compute just the unique block per core and let the output DMAs
duplicate it on the way to HBM.

Sharding: X (64) split across 8 cores -> 4 unique x2-values per core
(conv halos sliced host-side).  Per core, with partitions p=(x2,y2):

  out[t2,x2,y2,z2,c] = M1[p,(z2,c)] * ty[t2,y2,c] * Q[t2,x2,z2,c]
  M1 = uxy*uxz*uyz (pre-upsample conv outs),  Q = utx*utz.

All cross-partition broadcasts are done by tiny replicated DMA loads
from conv-output dumps in DRAM (0-stride partition dims), so the whole
voxel phase is a handful of VectorE tensor_tensor ops; no PE matmuls
outside the convolutions.  Each unique [128, 1024] f32 tile is stored
4x (t-dup x x-dup) with y/z duplication folded into the DMA access
patterns (4 KiB contiguous runs).
"""

import numpy as np

T2, X2, Y2, Z2, C = 8, 4, 32, 16, 16
NCORES = 8
CIN = 64

_CACHE = {}


def _build_program():
    from contextlib import ExitStack

    import concourse.bacc as bacc
    import concourse.bass as bass
    import concourse.mybir as mybir
    from concourse.tile import TileContext

    f32 = mybir.dt.float32
    AF = mybir.ActivationFunctionType
    MUL = mybir.AluOpType.mult
    AP = bass.AP

    nc = bacc.Bacc()
    ctx = ExitStack()

    # ---- external IO ----
    # One packed fp16 input: rows 0..63 = cin, row 64 = ones (bias channel).
    # Column segments: xyT[0:206] xz[206:316] yz[316:930] tx[930:992]
    # tyT[992:1334] tz[1334:1516] w[1516:2380]; convs read 3x3 windows,
    # w holds (plane, dy, dx, cout) with the bias in row 64 of the center tap.
    f16 = mybir.dt.float16
    KP = CIN + 1
    img_all = nc.dram_tensor("img_all", [KP, 2380], f16, kind="ExternalInput")
    out_d = nc.dram_tensor("out", [2 * T2, 2 * X2, 2 * Y2, 2 * Z2, C], f32,
                           kind="ExternalOutput")
    SEG = {"xyT": 0, "xz": 206, "yz": 316, "tx": 930, "tyT": 992,
           "tz": 1334, "w": 1516}

    # ---- DRAM scratch: raw conv-output dumps (flat [m*16]) ----
    yz_rows = [(0, 7), (7, 7), (14, 7), (21, 7), (28, 4)]
    ty_rows = [(0, 12), (12, 12), (24, 8)]
    edump = {}
    xy_rows = [(0, 21), (21, 11)]
    for k, m in ([("xz", 72), ("tx", 48), ("tz0", 72), ("tz1", 72)]
                 + [(f"xy{b}", nr * 6) for b, (r0, nr) in enumerate(xy_rows)]
                 + [(f"yz{b}", nr * 18) for b, (r0, nr) in enumerate(yz_rows)]
                 + [(f"ty{b}", nr * 10) for b, (r0, nr) in enumerate(ty_rows)]):
        edump[k] = nc.dram_tensor(f"e_{k}", [m * 16], f32)
    quD = nc.dram_tensor("quD", [32 * 256], f32)  # (x2, t2, z2, c) flat
    warmD = nc.dram_tensor("warmD", [16], f32)

    with TileContext(nc) as tc:
        sb = lambda name, shape: ctx.enter_context(
            nc.sbuf_tensor(name, shape, f32))
        # inputs (single packed fp16 tile)
        i_all = ctx.enter_context(nc.sbuf_tensor("i_all", [KP, 2380], f16))
        # voxel operands (partitions p = y2*4 + x2 unless noted)
        uxy_sb = sb("uxy_sb", [128, 16])      # p: c
        uxz_rep = sb("uxz_rep", [128, 256])   # p: (z2, c)  [rep over y2]
        uyz_rep = sb("uyz_rep", [128, 256])   # p: (z2, c)  [rep over x2]
        uty_rep = sb("uty_rep", [128, 128])   # p: (t2, c)  [rep over x2]
        qu_rep = sb("qu_rep", [128, 2048])    # p: (t2, z2, c)  [rep over y2]
        utx_sb = sb("utx_sb", [32, 16])       # p=(t2,x2): c
        utz_sb = sb("utz_sb", [32, 256])      # p=(t2,x2): (z2, c)
        qu_sb = sb("qu_sb", [32, 256])        # p=(t2,x2): (z2, c)
        m1a = sb("m1a", [128, 256])
        m1u = sb("m1u", [128, 256])
        tmp_all = sb("tmp_all", [128, 2048])  # p: (t2, z2, c) = m1u * ty

        # ---------- phase A: input load ----------
        nc.sync.dma_start(i_all[:], img_all[:])

        # ---------- PE warm-up (runs during the input DMA) ----------
        # The HAM clock gate keeps PE at 1.2 GHz until ~3.4us of sustained
        # activity; burn dummy matmuls so the convolutions run at 2.4 GHz.
        warm_sb = ctx.enter_context(nc.sbuf_tensor("warm_sb", [128, 512], f16))
        warm_out = sb("warm_out", [1, 16])
        nc.vector.memset(warm_sb[:], 0.0)
        with tc.tile_pool(name="warmpsum", bufs=2, space="PSUM") as wpool:
            wp_t = None
            for i in range(8):
                wp_t = wpool.tile([128, 512], f32, name=f"wp{i}", tag="wp")
                nc.tensor.matmul(wp_t, warm_sb[:, :128], warm_sb[:],
                                 start=True, stop=True)
            nc.scalar.activation(warm_out[:], wp_t[:1, :16], AF.Relu)
        nc.sync.dma_start(warmD[:], warm_out[:])

        # ---------- phase B: convolutions ----------
        def wslice(i, dy, dx):
            off = SEG["w"] + ((i * 3 + dy) * 3 + dx) * 16
            return AP(i_all, off, [[2380, KP], [1, 16]])

        conv_pool_cm = tc.tile_pool(name="convpsum", bufs=2, space="PSUM")
        conv_pool = conv_pool_cm.__enter__()

        conv_outs = {}

        def conv_spatial(i, seg, wp, rows, row0, tag):
            # Full-width contiguous windows; junk at cols wp-2, wp-1.
            m = rows * wp
            psum = conv_pool.tile([m, 16], f32, name=f"cp_{tag}", tag="cp")
            for dy in range(3):
                for dx in range(3):
                    lhsT = AP(i_all, SEG[seg] + (row0 + dy) * wp + dx,
                              [[2380, KP], [1, m]])
                    nc.tensor.matmul(psum, lhsT, wslice(i, dy, dx),
                                     start=(dy == 0 and dx == 0),
                                     stop=(dy == 2 and dx == 2))
            out_sb = sb(f"c_{tag}", [m, 16])
            nc.scalar.activation(out_sb[:], psum, AF.Relu)
            conv_outs[tag] = out_sb

        dump_insts = {}

        def dump(eng, k):
            dump_insts[k] = eng.dma_start(edump[k][:], conv_outs[k][:])

        def reload(eng, deps, dst_ap, src_ap):
            inst = eng.dma_start(dst_ap, src_ap)
            for d in deps:
                bass._add_dep_helper(inst.ins, dump_insts[d].ins,
                                     reason=f"raw {d}")
            return inst

        # --- Q path first: tx, tz, then ty ---
        conv_spatial(3, "tx", 6, 8, 0, "tx")               # m=48
        dump(nc.sync, "tx")
        for k in range(2):
            conv_spatial(5, "tz", 18, 4, 4 * k, f"tz{k}")  # m=72
            dump(nc.sync, f"tz{k}")
        for b, (r0, nr) in enumerate(ty_rows):
            conv_spatial(4, "tyT", 10, nr, r0, f"ty{b}")
            dump(nc.sync, f"ty{b}")

        # Q = utx * utz on p=(t2, x2)
        reload(nc.sync, ["tx"], utx_sb[:],
               AP(edump["tx"], 0, [[96, 8], [16, 4], [1, 16]]))
        for k in range(2):
            reload(nc.sync, [f"tz{k}"],
                   AP(utz_sb, k * 16 * 256, [[256, 16], [1, 256]]),
                   AP(edump[f"tz{k}"], 0, [[288, 4], [0, 4], [1, 256]]))
        nc.vector.tensor_tensor(
            qu_sb[:], utz_sb[:], AP(utx_sb, 0, [[16, 32], [0, 16], [1, 16]]),
            MUL)
        # store as (x2, t2, z2, c) so the replicated reload is contiguous
        qu_store = nc.sync.dma_start(
            AP(quD, 0, [[256, 8], [2048, 4], [1, 256]]), qu_sb[:])
        qu_load = nc.sync.dma_start(
            qu_rep[:], AP(quD, 0, [[0, 32], [2048, 4], [1, 2048]]))
        bass._add_dep_helper(qu_load.ins, qu_store.ins, reason="raw quD")

        # uty_rep[p=(y2,x2), (t2, c)] from transposed-ty dumps
        for b, (r0, nr) in enumerate(ty_rows):
            reload(nc.sync, [f"ty{b}"],
                   AP(uty_rep, r0 * 4 * 128, [[128, 4 * nr], [1, 128]]),
                   AP(edump[f"ty{b}"], 0, [[160, nr], [0, 4], [1, 128]]))

        # --- M1 path: xy (transposed plane -> y2-major rows), xz, yz ---
        for b, (r0, nr) in enumerate(xy_rows):
            conv_spatial(0, "xyT", 6, nr, r0, f"xy{b}")
            dump(nc.scalar, f"xy{b}")
            reload(nc.scalar, [f"xy{b}"],
                   AP(uxy_sb, r0 * 4 * 16, [[16, 4 * nr], [1, 16]]),
                   AP(edump[f"xy{b}"], 0, [[96, nr], [16, 4], [1, 16]]))

        conv_spatial(1, "xz", 18, 4, 0, "xz")             # m=72
        dump(nc.scalar, "xz")
        reload(nc.scalar, ["xz"], uxz_rep[:],
               AP(edump["xz"], 0, [[0, 32], [288, 4], [1, 256]]))
        for b, (r0, nr) in enumerate(yz_rows):
            conv_spatial(2, "yz", 18, nr, r0, f"yz{b}")
            dump(nc.scalar, f"yz{b}")
            reload(nc.scalar, [f"yz{b}"],
                   AP(uyz_rep, r0 * 4 * 256, [[256, 4 * nr], [1, 256]]),
                   AP(edump[f"yz{b}"], 0, [[288, nr], [0, 4], [1, 256]]))

        conv_pool_cm.__exit__(None, None, None)

        # ---------- phase C: M1 and ty products ----------
        nc.vector.tensor_tensor(m1a[:], uxz_rep[:], uyz_rep[:], MUL)
        nc.vector.tensor_tensor(
            m1u[:], m1a[:], AP(uxy_sb, 0, [[16, 128], [0, 16], [1, 16]]), MUL)
        # tmp_all[p, (t2, z2, c)] = m1u[p, (z2, c)] * uty_rep[p, (t2, c)]
        nc.vector.tensor_tensor(
            tmp_all[:],
            AP(m1u, 0, [[256, 128], [0, 8], [16, 16], [1, 16]]),
            AP(uty_rep, 0, [[128, 128], [16, 8], [0, 16], [1, 16]]), MUL)

        # ---------- phase D: per-t2 voxel tiles + duplicated stores ----------
        from contextlib import ExitStack as _ES
        pool_ctx = _ES()
        out_pool = pool_ctx.enter_context(tc.tile_pool(name="outsb", bufs=5))

        for t2 in range(T2):
            o = out_pool.tile([128, 1024], f32, name="o", tag="o")
            op = o.ap[0][0]
            # o[p, (z2, zd, c)] = tmp_all[p, t2, z2, c] * qu_rep[p, t2, z2, c]
            nc.vector.tensor_tensor(
                AP(o.tensor, o.offset, [[op, 128], [32, 16], [16, 2], [1, 16]]),
                AP(tmp_all, t2 * 256, [[2048, 128], [16, 16], [0, 2], [1, 16]]),
                AP(qu_rep, t2 * 256, [[2048, 128], [16, 16], [0, 2], [1, 16]]),
                MUL)
            # duplicate the (z, c) half-row for the y-duplication run
            nc.vector.tensor_copy(
                AP(o.tensor, o.offset + 512, [[op, 128], [1, 512]]),
                AP(o.tensor, o.offset, [[op, 128], [1, 512]]))
            for td in range(2):
                for xd in range(2):
                    eng = nc.sync if (td * 2 + xd) % 2 == 0 else nc.scalar
                    dst = AP(out_d,
                             (2 * t2 + td) * 262144 + xd * 32768,
                             [[1024, 32], [65536, 4], [1, 1024]])
                    eng.dma_start(dst, o[:])

        pool_ctx.close()

    nc.compile()
    return nc, ctx


def _prep_inputs(plane_xy, plane_xz, plane_yz, plane_tx, plane_ty, plane_tz,
                 W, b):
    """Host-side slicing/padding/transposition into one packed fp16 input."""
    f32 = np.float32
    xy = np.asarray(plane_xy, f32)[0]  # [64, X'32, Y'32]
    xz = np.asarray(plane_xz, f32)[0]  # [64, X'32, Z'16]
    yz = np.asarray(plane_yz, f32)[0]  # [64, Y'32, Z'16]
    tx = np.asarray(plane_tx, f32)[0]  # [64, T'8,  X'32]
    ty = np.asarray(plane_ty, f32)[0]  # [64, T'8,  Y'32]
    tz = np.asarray(plane_tz, f32)[0]  # [64, T'8,  Z'16]
    W = np.asarray(W, f32)             # [6, 16, 64, 3, 3]
    b = np.asarray(b, f32)             # [6, 16]

    # xy and ty are convolved on transposed planes -> swap their 3x3 taps
    W2 = W.copy()
    W2[0] = W[0].transpose(0, 1, 3, 2)
    W2[4] = W[4].transpose(0, 1, 3, 2)
    # weight block [65, 864]: rows 0..63 = (ci, i, dy, dx, co); row 64 holds
    # the bias in the center tap (the ones-channel contributes it once).
    wseg = np.zeros((65, 864), f32)
    wseg[:64] = W2.transpose(2, 0, 3, 4, 1).reshape(CIN, 864)
    for i in range(6):
        wseg[64, ((i * 3 + 1) * 3 + 1) * 16:((i * 3 + 1) * 3 + 1) * 16 + 16] = b[i]

    def flat2(p):
        q = p.reshape(p.shape[0], -1)
        return np.ascontiguousarray(np.pad(q, ((0, 0), (0, 2))))

    def with_ones(img):
        return np.concatenate([img, np.ones((1, img.shape[1]), f32)], axis=0)

    img_yz = flat2(np.pad(yz, ((0, 0), (1, 1), (1, 1))))
    img_tyT = flat2(np.pad(ty.transpose(0, 2, 1), ((0, 0), (1, 1), (1, 1))))
    img_tz = flat2(np.pad(tz, ((0, 0), (1, 1), (1, 1))))

    def row_halo(p, x0h):
        out = np.zeros((p.shape[0], 6, p.shape[2]), f32)
        lo = x0h - 1
        s0, s1 = max(lo, 0), min(lo + 6, p.shape[1])
        out[:, s0 - lo:s0 - lo + (s1 - s0), :] = p[:, s0:s1, :]
        return out

    def col_halo(p, x0h):
        out = np.zeros((p.shape[0], p.shape[1], 6), f32)
        lo = x0h - 1
        s0, s1 = max(lo, 0), min(lo + 6, p.shape[2])
        out[:, :, s0 - lo:s0 - lo + (s1 - s0)] = p[:, :, s0:s1]
        return out

    in_maps = []
    for k in range(NCORES):
        x0h = 4 * k
        segs = [
            flat2(np.pad(col_halo(xy.transpose(0, 2, 1), x0h),
                         ((0, 0), (1, 1), (0, 0)))),            # xyT 206
            flat2(np.pad(row_halo(xz, x0h), ((0, 0), (0, 0), (1, 1)))),  # 110
            img_yz,                                             # 614
            flat2(np.pad(col_halo(tx, x0h), ((0, 0), (1, 1), (0, 0)))),  # 62
            img_tyT,                                            # 342
            img_tz,                                             # 182
        ]
        img = np.concatenate([with_ones(s) for s in segs] + [wseg], axis=1)
        in_maps.append({"img_all": img.astype(np.float16)})
    return in_maps


def kernel(plane_xy, plane_xz, plane_yz, plane_tx, plane_ty, plane_tz, W, b):
    from concourse.bass_utils import run_bass_kernel_spmd

    if "nc" not in _CACHE:
        _CACHE["nc"], _CACHE["ctx"] = _build_program()
    nc = _CACHE["nc"]

    in_maps = _prep_inputs(plane_xy, plane_xz, plane_yz, plane_tx, plane_ty,
                           plane_tz, W, b)
    res = run_bass_kernel_spmd(nc, in_maps, list(range(NCORES)))
    slices = [res.results[k]["out"] for k in range(NCORES)]
    full = np.concatenate(slices, axis=1)  # [T, 64, Y, Z, C]
    return full[None].astype(np.float32)
